# revision 14
# baseline (speedup 1.0000x reference)
"""Trainium2 Bass kernel for masked causal multi-head attention.

Problem (hardcoded):
    x: (4, 2048, 512) f32, m: (4, 2048, 1) f32 (prefix 0/1 mask),
    w_qkv: (512, 1536) f32, w_out: (512, 512) f32, b_out: (512,) f32
    out = (softmax(mask(QK^T/8)) V) @ w_out + b_out, masked by m.

Sharding: 8 cores = 4 batches x 2 head-groups (4 heads each).  Each core
computes qkv projection for its (batch, head-group), flash-style causal
attention, and a partial out-projection; the host sums the two partials
per batch (replaces the all-reduce) and adds b_out.

Layout strategy (all compute bf16, accumulation f32 in PSUM):
  - x is passed pre-transposed per core: xt (512, L) where L = 128*nblk,
    nblk = ceil(maxlen/128) key/query blocks actually needed.
  - Q^T, K^T computed in (dh, t) layout, two heads stacked per 128
    partitions -> scores are computed transposed: S^T (k, q) tiles, which
    makes softmax need no transposes anywhere:
      * no max-subtraction (scores are ~N(0,1.2), |s| <= ~9; exp is safe
        in f32/bf16) -> no running max, plain PSUM accumulation
      * exp on the scalar engine, causal triangle handled by adding -30
        on the diagonal blocks before exp
      * row sums l come from an extra all-ones column appended to V
      * O^T = V_aug^T P accumulated over key blocks in PSUM
  - 1/l is broadcast across partitions with a tiny ones(1,64) matmul and
    applied during the PSUM->SBUF copy of O^T.
  - Out-projection contracts the two stacked heads per matmul and is
    masked by m per row during the PSUM->SBUF copy.
"""

import sys

import numpy as np

try:
    import concourse.bass as bass  # noqa: F401
except ImportError:  # pragma: no cover
    sys.path.insert(0, "/opt/trn_rl_repo")

import concourse.bacc as bacc
import concourse.mybir as mybir
import concourse.tile as tile
from concourse import bass_utils

F32 = mybir.dt.float32
BF16 = mybir.dt.bfloat16
NP_BF16 = mybir.dt.np(BF16)
AF = mybir.ActivationFunctionType

B, T, D, H = 4, 2048, 512, 8
DH = D // H  # 64
G = 2  # head groups (cores per batch)
HPG = H // G  # heads per group = 4
SCALE = DH**-0.5
NEG = -30.0  # masked-score additive constant; exp(-30) ~ 1e-13
N_CORES = 8


def build_nc(nblk: int):
    """Build the single SPMD Bass graph (same program on all 8 cores)."""
    L = nblk * 128
    NS = (L + 511) // 512  # number of 512-wide query superblocks

    def fs(s):  # query width of superblock s
        return min(512, L - 512 * s)

    def kbmax(s):  # causal+clamp bound on key blocks for superblock s
        return min(4 * s + (fs(s) + 127) // 128, nblk)

    nc = bacc.Bacc(
        "TRN2",
        target_bir_lowering=False,
        debug=False,
        enable_asserts=False,
        num_devices=N_CORES,
    )
    xt_d = nc.dram_tensor("xt", [D, L], BF16, kind="ExternalInput").ap()
    wq_d = nc.dram_tensor("wq", [2, 4, 128, 128], BF16, kind="ExternalInput").ap()
    wk_d = nc.dram_tensor("wk", [2, 4, 128, 128], BF16, kind="ExternalInput").ap()
    wv_d = nc.dram_tensor("wv", [4, 128, 256], BF16, kind="ExternalInput").ap()
    wo_d = nc.dram_tensor("wo", [2, 128, 512], BF16, kind="ExternalInput").ap()
    m_d = nc.dram_tensor("m", [128, nblk], F32, kind="ExternalInput").ap()
    tri_d = nc.dram_tensor("tri", [128, 128], BF16, kind="ExternalInput").ap()
    out_d = nc.dram_tensor("out", [T, D], BF16, kind="ExternalOutput").ap()

    with tile.TileContext(nc) as tc:
        with (
            tc.tile_pool(name="const", bufs=1) as cpool,
            tc.tile_pool(name="work", bufs=3) as wpool,
            tc.tile_pool(name="ps", bufs=2, space="PSUM") as pspool,
            tc.tile_pool(name="pwork", bufs=5) as ppool,
            tc.tile_pool(name="s_ps", bufs=2, space="PSUM") as spool,
            tc.tile_pool(name="o_ps", bufs=2, space="PSUM") as opool,
        ):
            # ---- persistent inputs -> SBUF ----
            # xt superblock chunks are fetched just-in-time: chunks 0/1
            # upfront (spread over four engines' DMA queues so the first
            # qkv isn't gated on one queue), chunk s+2 prefetched inside
            # build_qkv(s).
            dma_engines = [nc.sync, nc.scalar, nc.sync, nc.scalar]
            xt = [
                cpool.tile([128, L], BF16, tag=f"xt{d4}", name=f"xt{d4}")
                for d4 in range(4)
            ]

            def fetch_xt(s):
                if s >= NS:
                    return
                c0, w = 512 * s, fs(s)
                for d4 in range(4):
                    dma_engines[d4].dma_start(
                        xt[d4][:, c0 : c0 + w],
                        xt_d[128 * d4 : 128 * (d4 + 1), c0 : c0 + w],
                    )

            fetch_xt(0)
            fetch_xt(1)
            wq_sb = cpool.tile([128, 1024], BF16, tag="wq", name="wq_sb")
            wk_sb = cpool.tile([128, 1024], BF16, tag="wk", name="wk_sb")
            for hp in range(2):
                for d4 in range(4):
                    col = 128 * (4 * hp + d4)
                    nc.sync.dma_start(wq_sb[:, col : col + 128], wq_d[hp, d4])
                    nc.sync.dma_start(wk_sb[:, col : col + 128], wk_d[hp, d4])
            wv_sb = cpool.tile([128, 1024], BF16, tag="wv", name="wv_sb")
            for d4 in range(4):
                nc.sync.dma_start(wv_sb[:, 256 * d4 : 256 * (d4 + 1)], wv_d[d4])
            wo_sb = cpool.tile([128, 1024], BF16, tag="wo", name="wo_sb")
            for hp in range(2):
                nc.sync.dma_start(wo_sb[:, 512 * hp : 512 * (hp + 1)], wo_d[hp])
            m_sb = cpool.tile([128, nblk], F32, tag="m", name="m_sb")
            nc.sync.dma_start(m_sb[:], m_d[:])
            tri_sb = cpool.tile([128, 128], BF16, tag="tri", name="tri_sb")
            nc.sync.dma_start(tri_sb[:], tri_d[:])
            ones_sb = cpool.tile([1, 64], F32, tag="ones", name="ones_sb")
            nc.vector.memset(ones_sb[:], 1.0)

            # HAM warm-up: a dense burst of dummy full-array matmuls gated
            # on the first xt chunk, so it runs during the tail of the DMA
            # lead-in and the PE clock gate is at 8/8 when qkv(0) starts
            # (the gate needs ~3.4us of sustained activity; an early burst
            # followed by idle would just re-throttle).
            wu_sb = cpool.tile([128, 128], BF16, tag="wu", name="wu_sb")
            nc.vector.memset(wu_sb[:], 0.0)
            wu_ps = pspool.tile([128, 512], F32, tag="ps", name="wu_ps")
            for _ in range(60):
                nc.tensor.matmul(
                    wu_ps[:, :128],
                    lhsT=wu_sb[:],
                    rhs=xt[0][:, 0:128],
                    start=True,
                    stop=True,
                )

            # ---- qkv projections, built per superblock (see below) ----
            # Q^T and K^T: (dh, t) with the pair's two heads stacked on
            # partitions; V: (k, dh) per key block, 4 heads side by side,
            # each with an extra all-ones 65th column (row-sum trick).
            qt = {}
            kt = {}
            v = []

            def build_qkv(s):
                fetch_xt(s + 2)
                w = fs(s)
                for hp in range(2):
                    for which, wsb, store in (("q", wq_sb, qt), ("k", wk_sb, kt)):
                        ps = pspool.tile([128, 512], F32, tag="ps", name="ps")
                        for d4 in range(4):
                            col = 128 * (4 * hp + d4)
                            nc.tensor.matmul(
                                ps[:, :w],
                                lhsT=wsb[:, col : col + 128],
                                rhs=xt[d4][:, 512 * s : 512 * s + w],
                                start=(d4 == 0),
                                stop=(d4 == 3),
                            )
                        dst = cpool.tile([128, w], BF16, tag=f"{which}t{hp}_{s}", name=f"{which}t{hp}_{s}")
                        if which == "q":
                            # fold the attention scale into Q
                            nc.vector.tensor_scalar_mul(dst[:], ps[:, :w], SCALE)
                        else:
                            nc.vector.tensor_copy(dst[:], ps[:, :w])
                        store[(hp, s)] = dst
                for kb in range(4 * s, min(4 * s + (w + 127) // 128, nblk)):
                    ps = pspool.tile([128, 512], F32, tag="ps", name="ps")
                    for d4 in range(4):
                        nc.tensor.matmul(
                            ps[:, :256],
                            lhsT=xt[d4][:, 128 * kb : 128 * (kb + 1)],
                            rhs=wv_sb[:, 256 * d4 : 256 * (d4 + 1)],
                            start=(d4 == 0),
                            stop=(d4 == 3),
                        )
                    vt = cpool.tile([128, HPG * 65], BF16, tag=f"v{kb}", name=f"v{kb}")
                    v3 = vt[:].rearrange("p (h c) -> p h c", c=65)
                    nc.gpsimd.memset(v3[:, :, 64:65], 1.0)
                    nc.vector.tensor_copy(
                        v3[:, :, 0:64], ps[:, :256].rearrange("p (h c) -> p h c", c=64)
                    )
                    v.append(vt)

            # ---- attention + out-projection ----
            # Heads are processed in (h0, h1) pairs with a one-chunk
            # software pipeline (emit S(c) for both heads, then exp(c),
            # then AV(c-1)) so the exp latency is hidden behind the other
            # head's matmuls.  Each pair's normalize chain starts right at
            # pair end (frees the O PSUM slots fast), but the PE-side
            # broadcast matmul + final multiply are deferred until after
            # the next pair's first chunk; each superblock's out-projection
            # is deferred into the next superblock.  The PE stream
            # therefore never waits on the reciprocal chain.
            def plan_chunks(s):
                F = fs(s)
                KB = kbmax(s)
                chunks = []
                segs = []
                used = 0
                for kb in range(KB):
                    qoff = max(0, 128 * (kb - 4 * s))
                    feff = F - qoff
                    off = used
                    if off // 512 != (off + feff - 1) // 512:
                        off = 512 * ((off + 511) // 512)  # next bank
                    if off + feff > 1024:
                        chunks.append((segs, used))
                        segs = []
                        off = 0
                    segs.append((kb, qoff, feff, off))
                    used = off + feff
                if segs:
                    chunks.append((segs, used))
                return chunks

            def attention_pair(s, hp, o_ps2, ot_sb):
                F = fs(s)
                KB = kbmax(s)
                chunks = plan_chunks(s)
                done = [0, 0]

                def s_matmul(hi, out_ap, kb, qoff, feff):
                    p0 = 64 * hi
                    tck, off = divmod(kb, 4)
                    nc.tensor.matmul(
                        out_ap,
                        lhsT=kt[(hp, tck)][p0 : p0 + 64, 128 * off : 128 * off + 128],
                        rhs=qt[(hp, s)][p0 : p0 + 64, qoff : qoff + feff],
                        start=True,
                        stop=True,
                    )

                def do_avs(hi, segs, p_sb):
                    h = 2 * hp + hi
                    for kb, qoff, feff, off in segs:
                        nc.tensor.matmul(
                            o_ps2[hi][0:65, qoff : qoff + feff],
                            lhsT=v[kb][:, 65 * h : 65 * h + 65],
                            rhs=p_sb[:, off : off + feff],
                            start=(done[hi] == 0),
                            stop=(done[hi] == KB - 1),
                        )
                        done[hi] += 1

                prev = None  # (segs, [p_sb x2])
                for segs, used in chunks:
                    ps2 = []
                    for hi in range(2):
                        s_ps = spool.tile([128, 1024], F32, tag="s", name="s_ps")
                        for kb, qoff, feff, off in segs:
                            s_matmul(hi, s_ps[:, off : off + feff], kb, qoff, feff)
                        ps2.append(s_ps)
                    pb2 = []
                    for hi in range(2):
                        p_sb = ppool.tile([128, 1024], BF16, tag="p", name="p_sb")
                        nc.scalar.activation(p_sb[:, :used], ps2[hi][:, :used], AF.Exp)
                        for kb, qoff, feff, off in segs:
                            if kb >= 4 * s:
                                # diagonal: multiplicative causal triangle
                                nc.vector.tensor_mul(
                                    p_sb[:, off : off + 128],
                                    p_sb[:, off : off + 128],
                                    tri_sb[:],
                                )
                        pb2.append(p_sb)
                    if prev is not None:
                        for hi in range(2):
                            do_avs(hi, prev[0], prev[1][hi])
                    prev = (segs, pb2)
                for hi in range(2):
                    do_avs(hi, prev[0], prev[1][hi])

            def start_normalize(s, hp, o_ps2, ot_sb):
                """Fast part: drain o_ps and start the reciprocal chain.
                Returns the deferred finisher (PE broadcast + multiply)."""
                F = fs(s)
                nq = (F + 127) // 128
                rcs = []
                dsts = []
                for hi in range(2):
                    p0 = 64 * hi
                    dst = ot_sb[hp][p0 : p0 + 64, 0:F]
                    lrow = wpool.tile([1, 512], F32, tag="lrow", name="lrow")
                    nc.vector.tensor_copy(lrow[0:1, :F], o_ps2[hi][64:65, :F])
                    nc.scalar.activation(dst, o_ps2[hi][0:64, :F], AF.Copy)
                    # spread l across 128 partitions so the reciprocal uses
                    # all DVE lanes (2.75us -> 0.3us), then bring it back
                    lcol = wpool.tile([128, 4], F32, tag="lcol", name="lcol")
                    nc.gpsimd.dma_start(
                        lcol[:, 0:nq],
                        lrow[0:1, :F].rearrange("o (p c) -> o p c", c=nq),
                    )
                    rcol = wpool.tile([128, 4], F32, tag="rcol", name="rcol")
                    nc.vector.reciprocal(rcol[:, 0:nq], lcol[:, 0:nq])
                    rc = wpool.tile([1, 512], F32, tag="rc", name="rc")
                    nc.gpsimd.dma_start(
                        rc[0:1, :F].rearrange("o (p c) -> o p c", c=nq),
                        rcol[:, 0:nq],
                    )
                    rcs.append(rc)
                    dsts.append(dst)

                def finish():
                    for hi in range(2):
                        bc = pspool.tile([64, 512], F32, tag="ps", name="bc_ps")
                        nc.tensor.matmul(
                            bc[0:64, :F],
                            lhsT=ones_sb[0:1, :],
                            rhs=rcs[hi][0:1, :F],
                            start=True,
                            stop=True,
                        )
                        nc.vector.tensor_mul(dsts[hi], dsts[hi], bc[0:64, :F])

                return finish

            def make_outproj(s, ot_sb):
                def outproj():
                    F = fs(s)
                    for qi in range((F + 127) // 128):
                        y_ps = pspool.tile([128, 512], F32, tag="ps", name="ps")
                        for hp in range(2):
                            nc.tensor.matmul(
                                y_ps[:],
                                lhsT=ot_sb[hp][:, 128 * qi : 128 * (qi + 1)],
                                rhs=wo_sb[:, 512 * hp : 512 * (hp + 1)],
                                start=(hp == 0),
                                stop=(hp == 1),
                            )
                        ob = wpool.tile([128, 512], BF16, tag="ob", name="ob")
                        qg = 4 * s + qi
                        nc.vector.tensor_scalar_mul(
                            ob[:], y_ps[:], m_sb[:, qg : qg + 1]
                        )
                        row = 512 * s + 128 * qi
                        nc.sync.dma_start(out_d[row : row + 128, :], ob[:])

                return outproj

            pending_fin = None
            pending_out = None
            for s in range(NS):
                build_qkv(s)
                ot_sb = [
                    wpool.tile([128, 512], BF16, tag=f"ot{hp}", name=f"ot{hp}")
                    for hp in range(2)
                ]
                for hp in range(2):
                    o_ps2 = [
                        opool.tile([65, 512], F32, tag="o", name="o_ps")
                        for _ in range(2)
                    ]
                    attention_pair(s, hp, o_ps2, ot_sb)
                    if pending_fin is not None:
                        pending_fin()
                    pending_fin = start_normalize(s, hp, o_ps2, ot_sb)
                    if hp == 0 and pending_out is not None:
                        pending_out()
                        pending_out = None
                pending_out = make_outproj(s, ot_sb)
            pending_fin()
            pending_out()

    nc.compile()
    return nc


def make_in_maps(x, m, w_qkv, w_out, nblk: int):
    """Host-side sharding/packing: core c = (batch c//2, head-group c%2)."""
    L = nblk * 128
    tri = np.where(
        np.arange(128)[None, :] >= np.arange(128)[:, None], 1.0, 0.0
    ).astype(NP_BF16)
    in_maps = []
    for c in range(N_CORES):
        b, g = divmod(c, 2)
        xt = np.ascontiguousarray(x[b].T[:, :L]).astype(NP_BF16)
        wq = np.empty((2, 4, 128, 128), NP_BF16)
        wk = np.empty((2, 4, 128, 128), NP_BF16)
        for hp in range(2):
            for d4 in range(4):
                rows = slice(128 * d4, 128 * (d4 + 1))
                qcol = 256 * g + 128 * hp
                wq[hp, d4] = w_qkv[rows, qcol : qcol + 128]
                wk[hp, d4] = w_qkv[rows, 512 + qcol : 512 + qcol + 128]
        wv = np.empty((4, 128, 256), NP_BF16)
        for d4 in range(4):
            wv[d4] = w_qkv[128 * d4 : 128 * (d4 + 1), 1024 + 256 * g : 1024 + 256 * (g + 1)]
        wo = np.empty((2, 128, 512), NP_BF16)
        for hp in range(2):
            r0 = 256 * g + 128 * hp
            wo[hp] = w_out[r0 : r0 + 128, :]
        mp = np.ascontiguousarray(
            m[b, :L, 0].reshape(nblk, 128).T
        ).astype(np.float32)
        in_maps.append(
            {"xt": xt, "wq": wq, "wk": wk, "wv": wv, "wo": wo, "m": mp, "tri": tri}
        )
    return in_maps


def postprocess(results, x, m, b_out):
    out = np.zeros((B, T, D), np.float32)
    for b in range(B):
        out[b] = results[2 * b]["out"].astype(np.float32) + results[
            2 * b + 1
        ]["out"].astype(np.float32)
    out += b_out[None, None, :].astype(np.float32) * m.astype(np.float32)
    return out


def kernel(x, m, w_qkv, w_out, b_out):
    lengths = m[:, :, 0].astype(np.int64).sum(axis=1)
    nblk = max(1, int(-(-lengths.max() // 128)))
    nc = build_nc(nblk)
    in_maps = make_in_maps(x, m, w_qkv, w_out, nblk)
    res = bass_utils.run_bass_kernel_spmd(nc, in_maps, core_ids=list(range(N_CORES)))
    return postprocess(res.results, x, m, b_out)
